# revision 24
# baseline (speedup 1.0000x reference)
"""Trainium2 Bass kernel for nn_GatedAttn (gated attention with TISA bias).

Takes FULL inputs, returns FULL output. 8 NeuronCores, sharded as
(batch b = core//4) x (query-row slice r = core%4, 512 rows each); each core
runs the whole pipeline for its 512 query rows (K^T/V projections are
recomputed per core -- an AllGather variant that shares them across the
batch's 4 cores was measured SLOWER: the DRAM-DRAM collective exposes
~130us of latency that the saved PE time cannot cover).

Queries are processed in REVERSED order (host feeds xq columns backwards and
un-reverses output rows) so the per-k-block TISA bias factor is an ascending
contiguous slice of the eu table -- a descending slice would cost one 2-byte
DMA descriptor per element (5M packets = 5.6 ms, the original bottleneck).

DMA regime: dispatch instructions cost ~600ns on the issuing engine queue
and sub-1KB DRAM rows throttle the HWDGE queues, so ALL weights are
host-prepacked into wpack[128, 40960] in the exact SBUF tile layouts --
every weight load is one full-speed 2D DMA with 2-8KB contiguous rows.
xq/xT/EB ride the Scalar HWDGE queue, weights ride Sync, tiny denominator
row-moves ride the GpSimd SWDGE queue.

Per-core pipeline (all projection/attention matmuls in fp16 operands with
fp32 PSUM accumulation; rel err ~1.7e-3 vs the 2e-2 gate):
  startup:   Q proj (wq in 1024-col chunks) + V/K group-0 matmuls are
             emitted BEFORE the TISA selector matmuls so the PE starts
             ~10us in; the TISA DVE/ACT chain overlaps them. An early
             2-head selector pass produces eu rows 0:2 so pair 0's EB
             diagonal load isn't gated on the full TISA table.
  u-tables:  u[h,y] = sum_k amp*exp(-sh*(y-(511+512r+off))^2) via DVE
             shift/square + ACT Exp + an amplitude-selector matmul; eu =
             exp(u) (fp16) to DRAM; per head-pair load EB[p,x'] = eu[h,p+x']
             (all strides +1).
  attention: scores^T tiles (k_pos x q) via QK matmuls (contraction hd=64,
             head pairs at base partitions 0/64). Softmax without
             max-subtraction (|score| <= ~8.1): ACT exp (PSUM f32 -> SBUF
             fp16), DVE 2x-mode fp16 multiply with the EB table, fp16 AV
             matmuls; attn^T accumulates over 16 k-blocks in PSUM, row 64 =
             denominators. Denominator rows are staged (DVE copy + GpSimd
             SWDGE row-DMA) into den_sb partitions {2p+hi | p<6}, {32+hi |
             p=6}, {64+hi | p=7}; batched DVE reciprocals run at 32-aligned
             bases, CHUNKED 128 columns at a time across kb slots (a whole
             [12,512] reciprocal is 4.3us of in-order DVE queue that stalls
             the wt multiplies feeding the AV matmuls), so pairs 0..6
             normalize INSIDE the later pairs' kb loops and only pair 7's
             trails, overlapped with the first gate matmuls.
  gate:      (512 q x 2048) = out^T @ w_gate + b_gate (K=1 ones matmul) in
             four 4-target sub-phases (4 psum banks each, partial double
             buffering), a * sigmoid(g) -> fp16 (512, 1024) output slice,
             out DMAs alternating between the Sync and Scalar queues.

fp32r/fp16 PSUM-accumulation hazard: accumulating matmuls into a bank need
>=3 intervening matmuls -> all accumulation loops rotate >=4 bank targets.
"""

import sys
import os

for _p in ("/opt/trn_rl_repo", "/opt/pypackages"):
    if os.path.isdir(_p) and _p not in sys.path:
        sys.path.append(_p)

import numpy as np

import concourse.bass as bass
from concourse import bacc
import concourse.mybir as mybir
from concourse.tile import TileContext
from concourse.bass_utils import run_bass_kernel_spmd

F32 = mybir.dt.float32
F16 = mybir.dt.float16
F32R = mybir.dt.float32r
I32 = mybir.dt.int32
AF = mybir.ActivationFunctionType
MULT = mybir.AluOpType.mult
ADD = mybir.AluOpType.add

B, S, D = 2, 2048, 1024
H, NK, HD = 16, 21, 64
QS = 512
NCORES = 8
NPAIR = H // 2
NKB = S // 128
EBW = 2432
EUW = 2560
# wpack column offsets (see _pack_weights)
WQ0 = 0          # 8 chunks of 1024 (quad, dq)
WV0 = 8192       # 4 blocks of 2048 (group)
WK0 = 16384      # 8 blocks of 1024 (pair)
WG0 = 24576      # 4 blocks of 4096 (ph, ci)
WPACK_W = 40960


def build(debug=False):
    nc = bacc.Bacc("TRN2", target_bir_lowering=False, debug=False)

    xT_d = nc.dram_tensor("xT", [128, 16384], F16, kind="ExternalInput")
    xq_d = nc.dram_tensor("xq", [D, QS], F16, kind="ExternalInput")
    hd_d = nc.dram_tensor("hd", [128, 2048], F16, kind="ExternalInput")
    wpack_d = nc.dram_tensor("wpack", [128, WPACK_W], F16,
                             kind="ExternalInput")
    b_gate_d = nc.dram_tensor("b_gate", [1, 2 * D], F32R, kind="ExternalInput")
    eu_d = nc.dram_tensor("eu", [H, EUW], F16, kind="ExternalInput")
    ones_d = nc.dram_tensor("ones", [1, 128], F32R, kind="ExternalInput")
    sel_d = nc.dram_tensor("sel", [16, 1024], F32R, kind="ExternalInput")
    sel2_d = nc.dram_tensor("sel2", [2, 128], F32R, kind="ExternalInput")

    out_d = nc.dram_tensor("out", [QS, D], F16, kind="ExternalOutput")

    with TileContext(nc) as tc:
      with tc.tile_pool(name="gpool", bufs=1) as gpool:
        # All startup-critical loads ride ONE fast HWDGE queue (Scalar) in
        # priority order -- two concurrent queues share HBM arbitration
        # unevenly (the 4KB-row stream starves the small-row one ~8:1).
        # xq/xT/wq are host-prepacked so every row is 1-8KB contiguous.
        xqall = gpool.tile([128, 8 * QS], F16, name="xqall")
        xTall = gpool.tile([128, 8 * S], F16, name="xTall")
        # head [xq d0-1 | wq c0] is one host-fused dispatch: Q proj's first
        # 8 matmuls unblock on a single 512KB transfer
        hd_sb = gpool.tile([128, 2048], F16, name="hd_sb")
        nc.scalar.dma_start(hd_sb[:, :], hd_d[:, :])
        wqs = [hd_sb[:, 1024:2048]]

        def load_wq(i):
            wq = gpool.tile([128, 1024], F16, name=f"wq{i}")
            nc.scalar.dma_start(
                wq[:, :], wpack_d[:, WQ0 + 1024 * i:WQ0 + 1024 * i + 1024])
            wqs.append(wq)
        nc.scalar.dma_start(
            xqall[:, 1024:4096].rearrange("p (d c) -> p d c", d=6),
            bass.AP(xq_d, 256 * QS, [[QS, 128], [128 * QS, 6], [1, QS]]))
        for i in range(1, 4):
            load_wq(i)
        # xT kb-major (xp[p, 1024*kb + 128*d + c] = x[b, 128*kb+c, 128*d+p]):
        # V/K projections consume kb-chunks as they land.
        nc.scalar.dma_start(xTall[:, 0:4096], xT_d[:, 0:4096])
        for i in range(4, 8):
            load_wq(i)
        nc.scalar.dma_start(xTall[:, 4096:8192], xT_d[:, 4096:8192])
        # softmax denominator staging rows: pairs 0-5 at partitions 2p+hi,
        # pair 6 at 32+hi, pair 7 at 64+hi -- each reciprocal batch starts
        # at a 32-aligned partition.
        den_sb = gpool.tile([66, QS], F32, name="den_sb")
        rden7 = gpool.tile([2, QS], F32R, name="rden7")
        outT_sb = [gpool.tile([128, QS], F16, name=f"outT{p}")
                   for p in range(NPAIR)]
        wg_sb = [gpool.tile([128, 8192], F16, name=f"wg{ph}")
                 for ph in range(2)]
        ones_sb = gpool.tile([1, 128], F32R, name="ones_sb")
        sel_sb = gpool.tile([16, 1024], F32R, name="sel_sb")
        sel2_sb = gpool.tile([2, 128], F32R, name="sel2_sb")
        nc.scalar.dma_start(xTall[:, 8192:12288], xT_d[:, 8192:12288])
        nc.scalar.dma_start(xTall[:, 12288:16384], xT_d[:, 12288:16384])

        with tc.tile_pool(name="psum_o", bufs=2, space="PSUM") as pso:
          with tc.tile_pool(name="psum_m", bufs=3, space="PSUM") as psm:
            with tc.tile_pool(name="span", bufs=1) as span, \
                 tc.tile_pool(name="strm", bufs=2) as strm:

                # ---- Q^T proj (emitted right after the early eu01
                # pass so the PE starts as soon as xq d0 + wq chunk 0 land)
                qT_sb = [span.tile([128, QS], F16, name=f"qT{cb}")
                         for cb in range(8)]

                def emit_qproj():
                    for quad in range(2):
                        pq = [psm.tile([128, 1024], F32, name=f"pq{quad}{t}",
                                       tag="pm") for t in range(2)]
                        tgt = [pq[0][:, 0:512], pq[0][:, 512:1024],
                               pq[1][:, 0:512], pq[1][:, 512:1024]]
                        for dq in range(4):
                            wq = wqs[4 * quad + dq]
                            for d2 in range(2):
                                d = 2 * dq + d2
                                mv = (hd_sb[:, 512 * d:512 * d + 512]
                                      if d < 2 else
                                      xqall[:, 512 * d:512 * d + 512])
                                for t in range(4):
                                    nc.tensor.matmul(
                                        tgt[t],
                                        wq[:, 512 * d2 + 128 * t:
                                           512 * d2 + 128 * t + 128],
                                        mv,
                                        start=(d == 0), stop=(d == 7),
                                        skip_group_check=True)
                        for t in range(4):
                            nc.vector.tensor_copy(qT_sb[4 * quad + t][:, :],
                                                  tgt[t])

                # ---- attention tiles + JIT K/V proj ----
                v_sb = [[span.tile([128, 260], F16, name=f"v{gg}_{kb}",
                                   tag=f"v{gg % 2}_{kb}")
                         for kb in range(NKB)] for gg in range(4)]
                kp_pool = [span.tile([128, S], F16, name=f"kp{i}")
                           for i in range(2)]
                rden = span.tile([12, QS], F32R, name="rden")
                rden6 = span.tile([2, QS], F32R, name="rden6")

                def vproj_ops(g):
                    ops = []
                    wv = []

                    def load_wv():
                        t = strm.tile([128, 2048], F16, name="wv", tag="wv",
                                      bufs=2)
                        nc.sync.dma_start(
                            t[:, :],
                            wpack_d[:, WV0 + 2048 * g:WV0 + 2048 * g + 2048])
                        wv.append(t)
                    ops.append(load_wv)
                    for kq in range(4):
                        for tp in range(2):
                            def mkv(kq, tp):
                                def f():
                                    # one 512-wide bank: two 256-wide targets
                                    # (start=True clears the whole bank, so kb
                                    # tiles from different ops never share one)
                                    psv = psm.tile([128, 1024], F32,
                                                   name="psv", tag="pm")
                                    tg = [psv[:, 512 * t2:512 * t2 + 256]
                                          for t2 in range(2)]
                                    for d in range(8):
                                        for t2 in range(2):
                                            kb = 4 * kq + 2 * tp + t2
                                            nc.tensor.matmul(
                                                tg[t2],
                                                xTall[:, 1024 * kb + 128 * d:
                                                      1024 * kb + 128 * d + 128],
                                                wv[0][:, 256 * d:256 * d + 256],
                                                start=(d == 0),
                                                stop=(d == 7),
                                                skip_group_check=True)
                                    for t2 in range(2):
                                        kb = 4 * kq + 2 * tp + t2
                                        dst = v_sb[g][kb][:, :].rearrange(
                                            "p (j w) -> p j w", w=65)[:, :, 0:64]
                                        src = tg[t2].rearrange(
                                            "p (j w) -> p j w", w=64)
                                        # split between ACT and DVE queues so
                                        # neither stalls the wt multiplies
                                        if t2 == 0:
                                            nc.scalar.activation(dst, src,
                                                                 AF.Identity)
                                        else:
                                            nc.vector.tensor_copy(dst, src)
                                        oc = v_sb[g][kb][:, :].rearrange(
                                            "p (j w) -> p j w", w=65)[:, :, 64:65]
                                        nc.gpsimd.memset(oc, 1.0)
                                return f
                            ops.append(mkv(kq, tp))
                    return ops

                def kproj_ops(p):
                    ops = []
                    kp = kp_pool[p % 2]
                    wk = []

                    def load_wk():
                        t = strm.tile([128, 1024], F16, name="wk", tag="wk",
                                      bufs=2)
                        nc.sync.dma_start(
                            t[:, :],
                            wpack_d[:, WK0 + 1024 * p:WK0 + 1024 * p + 1024])
                        wk.append(t)
                    ops.append(load_wk)
                    psks = [None, None]
                    for half in range(2):
                        def mkk(half):
                            def f():
                                psk = psm.tile([128, 1024], F32, name="psk",
                                               tag="pm")
                                psks[half] = psk
                                xv = xTall[:, :].rearrange(
                                    "p (kb d c) -> p kb d c", kb=16, d=8)
                                for d in range(8):
                                    for t in range(2):
                                        kb0 = 8 * half + 4 * t
                                        nc.tensor.matmul(
                                            psk[:, 512 * t:512 * t + 512],
                                            wk[0][:, 128 * d:128 * d + 128],
                                            xv[:, kb0:kb0 + 4, d, :],
                                            start=(d == 0), stop=(d == 7),
                                            skip_group_check=True)
                                nc.vector.tensor_copy(
                                    kp[:, 1024 * half:1024 * half + 512],
                                    psk[:, 0:512])
                            return f

                        def cpk(half):
                            def f():
                                nc.scalar.activation(
                                    kp[:, 1024 * half + 512:
                                       1024 * half + 1024],
                                    psks[half][:, 512:1024], AF.Identity)
                            return f
                        ops.append(mkk(half))
                        ops.append(cpk(half))
                    return ops

                def norm_ops(q):
                    # normalize outT_sb[q] once its reciprocal batch is done
                    def f():
                        if q < 6:
                            sel_st = sel_sb[0:12, 128 * q:128 * q + 128]
                            mv = rden[:, :]
                        else:
                            sel_st = sel2_sb[:, :]
                            mv = rden6[:, :]
                        pbt = psm.tile([128, 1024], F32, name="pbn", tag="pm")
                        pb = pbt[:, 0:512]
                        nc.tensor.matmul(pb, sel_st, mv, start=True, stop=True)
                        nc.vector.tensor_tensor(
                            outT_sb[q][:, :], outT_sb[q][:, :], pb, MULT)
                    return f

                def recip_op(dst, src, ch):
                    # one 128-col chunk: a full [*,512] reciprocal is 4.3us
                    # of in-order DVE queue that stalls the wt multiplies
                    def f():
                        with nc.allow_low_precision(
                                reason="f32r reciprocal: 1.2e-4 rel is fine"):
                            nc.vector.reciprocal(dst[:, 128 * ch:128 * ch + 128],
                                                 src[:, 128 * ch:128 * ch + 128])
                    return f

                emit_qproj()

                # prologue: V group 0, K pair 0
                for op in vproj_ops(0):
                    op()
                for op in kproj_ops(0):
                    op()

                # selector tables (needed from the pair-5 normalization on;
                # issued after the early weight streams)
                nc.sync.dma_start(ones_sb[:, :], ones_d[:, :])
                nc.sync.dma_start(sel_sb[:, :], sel_d[:, :])
                nc.sync.dma_start(sel2_sb[:, :], sel2_d[:, :])

                for p in range(NPAIR):
                    hA = 2 * p
                    g, j0 = p // 2, 2 * (p % 2)
                    kp = kp_pool[p % 2]
                    eb = strm.tile([128, 2 * EBW], F16, name="eb", tag="eb",
                                   bufs=2)
                    # Queries run REVERSED (host feeds xq columns backwards),
                    # so the bias factor for score tile kb at [p, j] is
                    # eu[h, 128*kb + p + j]: load the diagonal table as
                    # eb[p, x'] = eu[h, p + x'] -- all strides +1 and
                    # contiguous (a -1 stride here costs one 2-byte DMA
                    # descriptor per element: 5M packets, 5.6 ms). One fused
                    # dispatch covers both heads.
                    nc.sync.dma_start(
                        eb[:, :].rearrange("p (i x) -> p i x", i=2),
                        bass.AP(eu_d, hA * EUW,
                                [[1, 128], [EUW, 2], [1, EBW]]))
                    if p == 1:
                        # gate-weight prefetch: after pair 0/1's eb loads so
                        # it doesn't block them, early enough to overlap.
                        # wg_sb[ph] layout: [p, ci*4096 + 512*d + c]
                        for ph in range(2):
                            for ci in range(2):
                                nc.sync.dma_start(
                                    wg_sb[ph][:, 4096 * ci:4096 * ci + 4096],
                                    wpack_d[:, WG0 + 4096 * (2 * ph + ci):
                                            WG0 + 4096 * (2 * ph + ci) + 4096])
                    pend = []
                    if p + 1 < NPAIR:
                        if (p + 1) % 2 == 0:
                            pend += vproj_ops((p + 1) // 2)
                        pend += kproj_ops(p + 1)
                    slots = [[] for _ in range(NKB)]
                    for i, op in enumerate(pend):
                        slots[min(1 + i, NKB - 1)].append(op)
                    if p == 6:
                        # pairs 0-5 dens are staged; batch reciprocal
                        # (4 chunks), then normalize pairs 0..2 here
                        for ch in range(4):
                            slots[3 + ch].append(
                                recip_op(rden, den_sb[0:12, :], ch))
                        slots[8].append(norm_ops(0))
                        slots[11].append(norm_ops(1))
                        slots[14].append(norm_ops(2))
                    if p == 7:
                        for ch in range(4):
                            slots[ch].append(
                                recip_op(rden6, den_sb[32:34, :], ch))
                        slots[2].append(norm_ops(3))
                        slots[5].append(norm_ops(4))
                        slots[8].append(norm_ops(5))
                        slots[13].append(norm_ops(6))

                    po = pso.tile([128, 512], F32, name="po", tag="po")[0:65, :]
                    po2 = pso.tile([128, 512], F32, name="po2",
                                   tag="po")[0:65, :]
                    for kb in range(NKB):
                        psc = psm.tile([128, 1024], F32, name="psc", tag="pm")
                        nc.tensor.matmul(psc[:, 0:512],
                                         kp[0:64, 128 * kb:128 * kb + 128],
                                         qT_sb[p][0:64, :], start=True,
                                         stop=True)
                        nc.tensor.matmul(psc[:, 512:1024],
                                         kp[64:128, 128 * kb:128 * kb + 128],
                                         qT_sb[p][64:128, :], start=True,
                                         stop=True)
                        et = strm.tile([128, 1024], F16, name="et", tag="et",
                                       bufs=3)
                        nc.scalar.activation(et[:, :], psc[:, :], AF.Exp,
                                             scale=0.125)
                        wt = strm.tile([128, 1024], F16, name="wt", tag="wt",
                                       bufs=3)
                        delta = 128 * kb
                        ebv = eb[:, :].rearrange("p (i x) -> p i x", i=2)[
                            :, :, delta:delta + 512]
                        nc.vector.tensor_tensor(
                            wt[:, :].rearrange("p (i q) -> p i q", i=2),
                            et[:, :].rearrange("p (i q) -> p i q", i=2),
                            ebv, MULT)
                        nc.tensor.matmul(
                            po[:, :], v_sb[g][kb][:, 65 * j0:65 * j0 + 65],
                            wt[:, 0:512], start=(kb == 0),
                            stop=(kb == NKB - 1), skip_group_check=True)
                        nc.tensor.matmul(
                            po2[:, :],
                            v_sb[g][kb][:, 65 * (j0 + 1):65 * (j0 + 1) + 65],
                            wt[:, 512:1024], start=(kb == 0),
                            stop=(kb == NKB - 1), skip_group_check=True)
                        for op in slots[kb]:
                            op()
                    for hi, pot in enumerate((po, po2)):
                        # stage the denominator row into its den_sb partition
                        # (DVE copy + GpSimd SWDGE row move: the Sync queue
                        # is busy with weight streams and DMA placement is
                        # the only way to change base partition).
                        dstg = strm.tile([1, 512], F32, name="dstg",
                                         tag="dstg")
                        nc.vector.tensor_copy(dstg[:, :], pot[64:65, :])
                        if p < 6:
                            drow = 2 * p + hi
                        elif p == 6:
                            drow = 32 + hi
                        else:
                            drow = 64 + hi
                        nc.gpsimd.dma_start(
                            den_sb[drow:drow + 1, :], dstg[:, :])
                        nc.vector.tensor_copy(
                            outT_sb[p][64 * hi:64 * hi + 64, :], pot[0:64, :])
          # psm and pso closed; gpool tiles stay live.
          # pair 7's reciprocal: the only one not overlapped by attention --
          # it runs while the gate's first d=0..6 matmuls stream.
        with nc.allow_low_precision(reason="f32r reciprocal"):
            nc.vector.reciprocal(rden7[:, :], den_sb[64:66, :])

        # ============ gate + GLU ============
        # four 4-target sub-phases (4 psum banks each; 3 tags double-
        # buffered so the next sub-phase's matmuls start while the previous
        # one drains). The first sub-phase's d=0..6 matmuls overlap pair 7's
        # reciprocal; its normalization lands before the d=7 contribution.
        with tc.tile_pool(name="gate", bufs=2) as gp, \
             tc.tile_pool(name="psum_g", bufs=1, space="PSUM") as psg:
            bg_sb = gp.tile([1, 2 * D], F32R, name="bg_sb", bufs=1)
            nc.sync.dma_start(bg_sb[:, :], b_gate_d[:, :])
            first = True
            oq = 0
            for ph in range(2):
                for qh in range(2):
                    qbs = (2 * qh, 2 * qh + 1)
                    pgt = {(qb, ci): psg.tile(
                               [128, 512], F32, name=f"pg{ph}{qb}{ci}",
                               tag=f"pg{qb % 2}{ci}",
                               bufs=(1 if (qb % 2, ci) == (1, 1) else 2))
                           for qb in qbs for ci in range(2)}
                    for d in range(7):
                        for qb in qbs:
                            for ci in range(2):
                                nc.tensor.matmul(
                                    pgt[(qb, ci)],
                                    outT_sb[d][:, 128 * qb:128 * qb + 128],
                                    wg_sb[ph][:, 4096 * ci + 512 * d:
                                              4096 * ci + 512 * d + 512],
                                    start=(d == 0), stop=False,
                                    skip_group_check=True)
                    if first:
                        first = False
                        pb7 = psg.tile([128, 512], F32, name="pb7", tag="pb7")
                        nc.tensor.matmul(pb7[:, :], sel2_sb[:, :],
                                         rden7[:, :], start=True, stop=True)
                        nc.vector.tensor_tensor(
                            outT_sb[7][:, :], outT_sb[7][:, :], pb7[:, :],
                            MULT)
                    for qb in qbs:
                        for ci in range(2):
                            nc.tensor.matmul(
                                pgt[(qb, ci)],
                                outT_sb[7][:, 128 * qb:128 * qb + 128],
                                wg_sb[ph][:, 4096 * ci + 512 * 7:
                                          4096 * ci + 512 * 7 + 512],
                                start=False, stop=False,
                                skip_group_check=True)
                    for qb in qbs:
                        for ci in range(2):
                            nc.tensor.matmul(
                                pgt[(qb, ci)], ones_sb[:, :],
                                bg_sb[:, 1024 * ci + 512 * ph:
                                      1024 * ci + 512 * ph + 512],
                                start=False, stop=True, skip_group_check=True)
                    for qb in qbs:
                        sg = gp.tile([128, 512], F32, name="sg", tag="sg")
                        nc.scalar.activation(sg[:, :], pgt[(qb, 1)],
                                             AF.Sigmoid)
                        res = gp.tile([128, 512], F16, name="res", tag="res")
                        nc.vector.tensor_tensor(res[:, :], pgt[(qb, 0)],
                                                sg[:, :], MULT)
                        eng = nc.sync if oq % 2 == 0 else nc.scalar
                        oq += 1
                        eng.dma_start(
                            out_d[128 * qb:128 * qb + 128,
                                  512 * ph:512 * ph + 512],
                            res[:, :])

    nc.finalize()
    return nc


_NC_CACHE = None
_LAST_IN_MAPS = None


def _get_nc():
    global _NC_CACHE
    if _NC_CACHE is None:
        _NC_CACHE = build()
    return _NC_CACHE


def _pack_weights(w_in16, w_gate16):
    """Prepack w_in/w_gate into the exact SBUF tile layouts so every
    weight load is one full-speed contiguous-row 2D DMA."""
    wp = np.empty((128, WPACK_W), np.float16)
    # wq chunk (quad, dq): [p, 512*d2 + c] = w_in[256*dq*?? rows]
    for quad in range(2):
        for dq in range(4):
            blk = w_in16[256 * dq:256 * dq + 256,
                         2 * D + 512 * quad:2 * D + 512 * quad + 512]
            # rows (2, 128) x cols 512 -> [128, 2, 512]
            wp[:, WQ0 + 1024 * (4 * quad + dq):
               WQ0 + 1024 * (4 * quad + dq) + 1024] = \
                blk.reshape(2, 128, 512).transpose(1, 0, 2).reshape(128, 1024)
    for g in range(4):
        blk = w_in16[:, D + 256 * g:D + 256 * g + 256]
        wp[:, WV0 + 2048 * g:WV0 + 2048 * g + 2048] = \
            blk.reshape(8, 128, 256).transpose(1, 0, 2).reshape(128, 2048)
    for pp in range(8):
        blk = w_in16[:, 128 * pp:128 * pp + 128]
        wp[:, WK0 + 1024 * pp:WK0 + 1024 * pp + 1024] = \
            blk.reshape(8, 128, 128).transpose(1, 0, 2).reshape(128, 1024)
    for ph in range(2):
        for ci in range(2):
            blk = w_gate16[:, 1024 * ci + 512 * ph:1024 * ci + 512 * ph + 512]
            wp[:, WG0 + 4096 * (2 * ph + ci):
               WG0 + 4096 * (2 * ph + ci) + 4096] = \
                blk.reshape(8, 128, 512).transpose(1, 0, 2).reshape(128, 4096)
    return wp


def kernel(x, w_in, w_gate, b_gate, amplitudes, sharpness, offsets):
    x = np.ascontiguousarray(x, dtype=np.float32)
    w_in16 = np.ascontiguousarray(w_in, dtype=np.float16)
    w_gate16 = np.ascontiguousarray(w_gate, dtype=np.float16)
    b_gate = np.ascontiguousarray(b_gate, dtype=np.float32).reshape(1, 2 * D)
    amplitudes = np.asarray(amplitudes, dtype=np.float32)
    sharpness = np.asarray(sharpness, dtype=np.float32)
    offsets = np.asarray(offsets, dtype=np.float32)

    wpack = _pack_weights(w_in16, w_gate16)
    # TISA eu table computed on host (tiny O(H*K*EUW) work): eu[h, y] =
    # exp(sum_k amp * exp(-sh * (y - (off + 511 + 512r))^2)), one table per
    # query-slice index r.
    y = np.arange(EUW, dtype=np.float32)
    eus = []
    for r in range(4):
        ctr = offsets[:, :, None] + (511.0 + 512.0 * r)
        dd = y[None, None, :] - ctr
        u = (amplitudes[:, :, None] *
             np.exp(-np.abs(sharpness)[:, :, None] * dd * dd)).sum(axis=1)
        eus.append(np.exp(u).astype(np.float16))
    ones = np.ones((1, 128), np.float32)
    # sel[r, 128p + c] = 1 iff r == 2p + c//64 (head selector used to
    # broadcast the batched softmax reciprocals to 128 output rows per pair)
    sel = np.zeros((16, 1024), np.float32)
    for p_ in range(8):
        sel[2 * p_, 128 * p_:128 * p_ + 64] = 1.0
        sel[2 * p_ + 1, 128 * p_ + 64:128 * p_ + 128] = 1.0
    # sel2[r, c] = 1 iff r == c//64 (2-row selector for pairs 6/7)
    sel2 = np.zeros((2, 128), np.float32)
    sel2[0, 0:64] = 1.0
    sel2[1, 64:128] = 1.0

    in_maps = []
    for c in range(NCORES):
        b, r = c // 4, c % 4
        # kb-major pack: xT[p, 1024*kb + 128*d + c] = x[b, 128*kb+c, 128*d+p]
        xT = np.ascontiguousarray(
            x[b].reshape(16, 128, 8, 128).transpose(3, 0, 2, 1)
            .reshape(128, 16384), dtype=np.float16)
        # query columns fed in REVERSED order so the TISA bias slice per
        # k-block is an ascending (contiguous-DMA) slice of the eu table;
        # the output rows are un-reversed after the run.
        xq = np.ascontiguousarray(x[b, QS * r:QS * r + QS, :].T[:, ::-1],
                                  dtype=np.float16)
        hd = np.empty((128, 2048), np.float16)
        hd[:, 0:1024] = (xq[0:256, :].reshape(2, 128, QS)
                         .transpose(1, 0, 2).reshape(128, 1024))
        hd[:, 1024:2048] = wpack[:, WQ0:WQ0 + 1024]
        in_maps.append({
            "xT": xT, "xq": xq, "hd": hd, "wpack": wpack, "b_gate": b_gate,
            "eu": eus[r], "ones": ones, "sel": sel, "sel2": sel2,
        })

    global _LAST_IN_MAPS
    _LAST_IN_MAPS = in_maps
    nc = _get_nc()
    r_ = run_bass_kernel_spmd(nc, in_maps, core_ids=list(range(NCORES)))
    out = np.empty((B, S, D), np.float32)
    for c in range(NCORES):
        b, r = c // 4, c % 4
        out[b, QS * r:QS * r + QS, :] = \
            r_.results[c]["out"][::-1, :].astype(np.float32)
    return out


# revision 25
# speedup vs baseline: 1.1967x; 1.1967x over previous
"""Trainium2 Bass kernel for nn_GatedAttn (gated attention with TISA bias).

Takes FULL inputs, returns FULL output. 8 NeuronCores, sharded as
(batch b = core//4) x (query-row slice r = core%4, 512 rows each); each core
runs the whole pipeline for its 512 query rows (K^T/V projections are
recomputed per core -- an AllGather variant that shares them across the
batch's 4 cores was measured SLOWER: the DRAM-DRAM collective exposes
~130us of latency that the saved PE time cannot cover).

Queries are processed in REVERSED order (host feeds xq columns backwards and
un-reverses output rows) so the per-k-block TISA bias factor is an ascending
contiguous slice of the eu table -- a descending slice would cost one 2-byte
DMA descriptor per element (5M packets = 5.6 ms, the original bottleneck).

DMA regime: dispatch instructions cost ~600ns on the issuing engine queue
and sub-1KB DRAM rows throttle the HWDGE queues, so ALL weights are
host-prepacked into wpack[128, 40960] in the exact SBUF tile layouts --
every weight load is one full-speed 2D DMA with 2-8KB contiguous rows.
xq/xT/EB ride the Scalar HWDGE queue, weights ride Sync, tiny denominator
row-moves ride the GpSimd SWDGE queue.

Per-core pipeline (all projection/attention matmuls in fp16 operands with
fp32 PSUM accumulation; rel err ~1.7e-3 vs the 2e-2 gate):
  startup:   Q proj (wq in 1024-col chunks) + V/K group-0 matmuls are
             emitted BEFORE the TISA selector matmuls so the PE starts
             ~10us in; the TISA DVE/ACT chain overlaps them. An early
             2-head selector pass produces eu rows 0:2 so pair 0's EB
             diagonal load isn't gated on the full TISA table.
  u-tables:  u[h,y] = sum_k amp*exp(-sh*(y-(511+512r+off))^2) via DVE
             shift/square + ACT Exp + an amplitude-selector matmul; eu =
             exp(u) (fp16) to DRAM; per head-pair load EB[p,x'] = eu[h,p+x']
             (all strides +1).
  attention: scores^T tiles (k_pos x q) via QK matmuls (contraction hd=64,
             head pairs at base partitions 0/64). Softmax without
             max-subtraction (|score| <= ~8.1): ACT exp (PSUM f32 -> SBUF
             fp16), DVE 2x-mode fp16 multiply with the EB table, fp16 AV
             matmuls; attn^T accumulates over 16 k-blocks in PSUM, row 64 =
             denominators. Denominator rows are staged (DVE copy + GpSimd
             SWDGE row-DMA) into den_sb partitions {2p+hi | p<6}, {32+hi |
             p=6}, {64+hi | p=7}; batched DVE reciprocals run at 32-aligned
             bases, CHUNKED 128 columns at a time across kb slots (a whole
             [12,512] reciprocal is 4.3us of in-order DVE queue that stalls
             the wt multiplies feeding the AV matmuls), so pairs 0..6
             normalize INSIDE the later pairs' kb loops and only pair 7's
             trails, overlapped with the first gate matmuls.
  gate:      (512 q x 2048) = out^T @ w_gate + b_gate (K=1 ones matmul) in
             four 4-target sub-phases (4 psum banks each, partial double
             buffering), a * sigmoid(g) -> fp16 (512, 1024) output slice,
             out DMAs alternating between the Sync and Scalar queues.

fp32r/fp16 PSUM-accumulation hazard: accumulating matmuls into a bank need
>=3 intervening matmuls -> all accumulation loops rotate >=4 bank targets.
"""

import sys
import os

for _p in ("/opt/trn_rl_repo", "/opt/pypackages"):
    if os.path.isdir(_p) and _p not in sys.path:
        sys.path.append(_p)

import numpy as np

import concourse.bass as bass
from concourse import bacc
import concourse.mybir as mybir
from concourse.tile import TileContext
from concourse.bass_utils import run_bass_kernel_spmd

F32 = mybir.dt.float32
F16 = mybir.dt.float16
F32R = mybir.dt.float32r
I32 = mybir.dt.int32
AF = mybir.ActivationFunctionType
MULT = mybir.AluOpType.mult
ADD = mybir.AluOpType.add

B, S, D = 2, 2048, 1024
H, NK, HD = 16, 21, 64
QS = 512
NCORES = 8
NPAIR = H // 2
NKB = S // 128
EBW = 2432
EUW = 2560
# wpack column offsets (see _pack_weights)
WQ0 = 0          # 8 chunks of 1024 (quad, dq)
WV0 = 8192       # 4 blocks of 2048 (group)
WK0 = 16384      # 8 blocks of 1024 (pair)
WG0 = 24576      # 4 blocks of 4096 (ph, ci)
WPACK_W = 40960


def build(debug=False):
    nc = bacc.Bacc("TRN2", target_bir_lowering=False, debug=False)

    xT_d = nc.dram_tensor("xT", [128, 16384], F16, kind="ExternalInput")
    xq_d = nc.dram_tensor("xq", [D, QS], F16, kind="ExternalInput")
    hd_d = nc.dram_tensor("hd", [128, 2048], F16, kind="ExternalInput")
    wpack_d = nc.dram_tensor("wpack", [128, WPACK_W], F16,
                             kind="ExternalInput")
    b_gate_d = nc.dram_tensor("b_gate", [1, 2 * D], F32R, kind="ExternalInput")
    eu_d = nc.dram_tensor("eu", [H, EUW], F16, kind="ExternalInput")
    ones_d = nc.dram_tensor("ones", [1, 128], F32R, kind="ExternalInput")
    sel_d = nc.dram_tensor("sel", [16, 1024], F32R, kind="ExternalInput")
    sel2_d = nc.dram_tensor("sel2", [2, 128], F32R, kind="ExternalInput")

    out_d = nc.dram_tensor("out", [QS, D], F16, kind="ExternalOutput")

    with TileContext(nc) as tc:
      with tc.tile_pool(name="gpool", bufs=1) as gpool:
        # All startup-critical loads ride ONE fast HWDGE queue (Scalar) in
        # priority order -- two concurrent queues share HBM arbitration
        # unevenly (the 4KB-row stream starves the small-row one ~8:1).
        # xq/xT/wq are host-prepacked so every row is 1-8KB contiguous.
        xqall = gpool.tile([128, 8 * QS], F16, name="xqall")
        xTall = gpool.tile([128, 8 * S], F16, name="xTall")
        # head [xq d0-1 | wq c0] is one host-fused dispatch: Q proj's first
        # 8 matmuls unblock on a single 512KB transfer
        hd_sb = gpool.tile([128, 2048], F16, name="hd_sb")
        nc.scalar.dma_start(hd_sb[:, :], hd_d[:, :])
        wqs = [hd_sb[:, 1024:2048]]

        def load_wq(i):
            wq = gpool.tile([128, 1024], F16, name=f"wq{i}")
            nc.scalar.dma_start(
                wq[:, :], wpack_d[:, WQ0 + 1024 * i:WQ0 + 1024 * i + 1024])
            wqs.append(wq)
        nc.scalar.dma_start(
            xqall[:, 1024:4096].rearrange("p (d c) -> p d c", d=6),
            bass.AP(xq_d, 256 * QS, [[QS, 128], [128 * QS, 6], [1, QS]]))
        for i in range(1, 4):
            load_wq(i)
        # xT kb-major (xp[p, 1024*kb + 128*d + c] = x[b, 128*kb+c, 128*d+p]):
        # V/K projections consume kb-chunks as they land.
        nc.scalar.dma_start(xTall[:, 0:4096], xT_d[:, 0:4096])
        for i in range(4, 8):
            load_wq(i)
        nc.scalar.dma_start(xTall[:, 4096:8192], xT_d[:, 4096:8192])
        # softmax denominator staging rows: pairs 0-5 at partitions 2p+hi,
        # pair 6 at 32+hi, pair 7 at 64+hi -- each reciprocal batch starts
        # at a 32-aligned partition.
        den_sb = gpool.tile([66, QS], F32, name="den_sb")
        rden7 = gpool.tile([2, QS], F32R, name="rden7")
        outT_sb = [gpool.tile([128, QS], F16, name=f"outT{p}")
                   for p in range(NPAIR)]
        wg_sb = [gpool.tile([128, 8192], F16, name=f"wg{ph}")
                 for ph in range(2)]
        ones_sb = gpool.tile([1, 128], F32R, name="ones_sb")
        sel_sb = gpool.tile([16, 1024], F32R, name="sel_sb")
        sel2_sb = gpool.tile([2, 128], F32R, name="sel2_sb")
        nc.scalar.dma_start(xTall[:, 8192:12288], xT_d[:, 8192:12288])
        nc.scalar.dma_start(xTall[:, 12288:16384], xT_d[:, 12288:16384])

        with tc.tile_pool(name="psum_o", bufs=2, space="PSUM") as pso:
          with tc.tile_pool(name="psum_m", bufs=3, space="PSUM") as psm:
            with tc.tile_pool(name="span", bufs=1) as span, \
                 tc.tile_pool(name="strm", bufs=2) as strm:

                # ---- Q^T proj (emitted right after the early eu01
                # pass so the PE starts as soon as xq d0 + wq chunk 0 land)
                qT_sb = [span.tile([128, QS], F16, name=f"qT{cb}")
                         for cb in range(8)]

                def emit_qproj():
                    for quad in range(2):
                        pq = [psm.tile([128, 1024], F32, name=f"pq{quad}{t}",
                                       tag="pm") for t in range(2)]
                        tgt = [pq[0][:, 0:512], pq[0][:, 512:1024],
                               pq[1][:, 0:512], pq[1][:, 512:1024]]
                        for dq in range(4):
                            wq = wqs[4 * quad + dq]
                            for d2 in range(2):
                                d = 2 * dq + d2
                                mv = (hd_sb[:, 512 * d:512 * d + 512]
                                      if d < 2 else
                                      xqall[:, 512 * d:512 * d + 512])
                                for t in range(4):
                                    nc.tensor.matmul(
                                        tgt[t],
                                        wq[:, 512 * d2 + 128 * t:
                                           512 * d2 + 128 * t + 128],
                                        mv,
                                        start=(d == 0), stop=(d == 7),
                                        skip_group_check=True)
                        for t in range(4):
                            nc.vector.tensor_copy(qT_sb[4 * quad + t][:, :],
                                                  tgt[t])

                # ---- attention tiles + JIT K/V proj ----
                v_sb = [[span.tile([128, 260], F16, name=f"v{gg}_{kb}",
                                   tag=f"v{gg % 2}_{kb}")
                         for kb in range(NKB)] for gg in range(4)]
                kp_pool = [span.tile([128, S], F16, name=f"kp{i}")
                           for i in range(2)]
                rden = span.tile([12, QS], F32R, name="rden")
                rden6 = span.tile([2, QS], F32R, name="rden6")

                def vproj_ops(g):
                    wv = []

                    def load_wv():
                        t = strm.tile([128, 2048], F16, name="wv", tag="wv",
                                      bufs=2)
                        nc.sync.dma_start(
                            t[:, :],
                            wpack_d[:, WV0 + 2048 * g:WV0 + 2048 * g + 2048])
                        wv.append(t)

                    psvs = {}

                    def mkv(kq, tp):
                        def f():
                            # one 512-wide bank: two 256-wide targets
                            # (start=True clears the whole bank, so kb tiles
                            # from different ops never share one)
                            psv = psm.tile([128, 1024], F32, name="psv",
                                           tag="pm")
                            psvs[(kq, tp)] = psv
                            for d in range(8):
                                for t2 in range(2):
                                    kb = 4 * kq + 2 * tp + t2
                                    nc.tensor.matmul(
                                        psv[:, 512 * t2:512 * t2 + 256],
                                        xTall[:, 1024 * kb + 128 * d:
                                              1024 * kb + 128 * d + 128],
                                        wv[0][:, 256 * d:256 * d + 256],
                                        start=(d == 0), stop=(d == 7),
                                        skip_group_check=True)
                        return f

                    def cpv(kq, tp):
                        # emitted 2 slots after mkv so the DVE queue never
                        # parks on an unfinished V projection
                        def f():
                            psv = psvs[(kq, tp)]
                            for t2 in range(2):
                                kb = 4 * kq + 2 * tp + t2
                                dst = v_sb[g][kb][:, :].rearrange(
                                    "p (j w) -> p j w", w=65)[:, :, 0:64]
                                nc.vector.tensor_copy(
                                    dst,
                                    psv[:, 512 * t2:512 * t2 + 256].rearrange(
                                        "p (j w) -> p j w", w=64))
                                oc = v_sb[g][kb][:, :].rearrange(
                                    "p (j w) -> p j w", w=65)[:, :, 64:65]
                                nc.gpsimd.memset(oc, 1.0)
                        return f

                    units = [(mkv(kq, tp), cpv(kq, tp))
                             for kq in range(4) for tp in range(2)]
                    ops = [load_wv]
                    pending = []
                    for mm, cp in units:
                        ops.append(mm)
                        pending.append(cp)
                        if len(pending) >= 2:
                            ops.append(pending.pop(0))
                    ops.extend(pending)
                    return ops

                def kproj_ops(p):
                    kp = kp_pool[p % 2]
                    wk = []

                    def load_wk():
                        t = strm.tile([128, 1024], F16, name="wk", tag="wk",
                                      bufs=2)
                        nc.sync.dma_start(
                            t[:, :],
                            wpack_d[:, WK0 + 1024 * p:WK0 + 1024 * p + 1024])
                        wk.append(t)

                    psks = [None, None]

                    def mkk(half):
                        def f():
                            psk = psm.tile([128, 1024], F32, name="psk",
                                           tag="pm")
                            psks[half] = psk
                            xv = xTall[:, :].rearrange(
                                "p (kb d c) -> p kb d c", kb=16, d=8)
                            for d in range(8):
                                for t in range(2):
                                    kb0 = 8 * half + 4 * t
                                    nc.tensor.matmul(
                                        psk[:, 512 * t:512 * t + 512],
                                        wk[0][:, 128 * d:128 * d + 128],
                                        xv[:, kb0:kb0 + 4, d, :],
                                        start=(d == 0), stop=(d == 7),
                                        skip_group_check=True)
                        return f

                    def cpk(half, piece):
                        def f():
                            nc.vector.tensor_copy(
                                kp[:, 1024 * half + 512 * piece:
                                   1024 * half + 512 * piece + 512],
                                psks[half][:, 512 * piece:512 * piece + 512])
                        return f

                    return [load_wk, mkk(0), mkk(1), cpk(0, 0), cpk(0, 1),
                            cpk(1, 0), cpk(1, 1)]

                def norm_ops(q):
                    # normalize outT_sb[q] once its reciprocal batch is done
                    def f():
                        if q < 6:
                            sel_st = sel_sb[0:12, 128 * q:128 * q + 128]
                            mv = rden[:, :]
                        else:
                            sel_st = sel2_sb[:, :]
                            mv = rden6[:, :]
                        pbt = psm.tile([128, 1024], F32, name="pbn", tag="pm")
                        pb = pbt[:, 0:512]
                        nc.tensor.matmul(pb, sel_st, mv, start=True, stop=True)
                        nc.vector.tensor_tensor(
                            outT_sb[q][:, :], outT_sb[q][:, :], pb, MULT)
                    return f

                def recip_op(dst, src, ch):
                    # one 128-col chunk: a full [*,512] reciprocal is 4.3us
                    # of in-order DVE queue that stalls the wt multiplies
                    def f():
                        with nc.allow_low_precision(
                                reason="f32r reciprocal: 1.2e-4 rel is fine"):
                            nc.vector.reciprocal(dst[:, 128 * ch:128 * ch + 128],
                                                 src[:, 128 * ch:128 * ch + 128])
                    return f

                emit_qproj()

                # prologue: V group 0, K pair 0
                for op in vproj_ops(0):
                    op()
                for op in kproj_ops(0):
                    op()

                # selector tables (needed from the pair-5 normalization on;
                # issued after the early weight streams)
                nc.sync.dma_start(ones_sb[:, :], ones_d[:, :])
                nc.sync.dma_start(sel_sb[:, :], sel_d[:, :])
                nc.sync.dma_start(sel2_sb[:, :], sel2_d[:, :])

                for p in range(NPAIR):
                    hA = 2 * p
                    g, j0 = p // 2, 2 * (p % 2)
                    kp = kp_pool[p % 2]
                    eb = strm.tile([128, 2 * EBW], F16, name="eb", tag="eb",
                                   bufs=2)
                    # Queries run REVERSED (host feeds xq columns backwards),
                    # so the bias factor for score tile kb at [p, j] is
                    # eu[h, 128*kb + p + j]: load the diagonal table as
                    # eb[p, x'] = eu[h, p + x'] -- all strides +1 and
                    # contiguous (a -1 stride here costs one 2-byte DMA
                    # descriptor per element: 5M packets, 5.6 ms). One fused
                    # dispatch covers both heads.
                    nc.sync.dma_start(
                        eb[:, :].rearrange("p (i x) -> p i x", i=2),
                        bass.AP(eu_d, hA * EUW,
                                [[1, 128], [EUW, 2], [1, EBW]]))
                    if p == 1:
                        # gate-weight prefetch: after pair 0/1's eb loads so
                        # it doesn't block them, early enough to overlap.
                        # wg_sb[ph] layout: [p, ci*4096 + 512*d + c]
                        for ph in range(2):
                            for ci in range(2):
                                nc.sync.dma_start(
                                    wg_sb[ph][:, 4096 * ci:4096 * ci + 4096],
                                    wpack_d[:, WG0 + 4096 * (2 * ph + ci):
                                            WG0 + 4096 * (2 * ph + ci) + 4096])
                    pend = []
                    if p + 1 < NPAIR:
                        if (p + 1) % 2 == 0:
                            pend += vproj_ops((p + 1) // 2)
                        pend += kproj_ops(p + 1)
                    slots = [[] for _ in range(NKB)]
                    for i, op in enumerate(pend):
                        slots[min(1 + i, NKB - 1)].append(op)
                    if p == 6:
                        # pairs 0-5 dens are staged; batch reciprocal
                        # (4 chunks), then normalize pairs 0..2 here
                        for ch in range(4):
                            slots[3 + ch].append(
                                recip_op(rden, den_sb[0:12, :], ch))
                        slots[8].append(norm_ops(0))
                        slots[11].append(norm_ops(1))
                        slots[14].append(norm_ops(2))
                    if p == 7:
                        for ch in range(4):
                            slots[ch].append(
                                recip_op(rden6, den_sb[32:34, :], ch))
                        slots[2].append(norm_ops(3))
                        slots[5].append(norm_ops(4))
                        slots[8].append(norm_ops(5))
                        slots[13].append(norm_ops(6))

                    po = pso.tile([128, 512], F32, name="po", tag="po")[0:65, :]
                    po2 = pso.tile([128, 512], F32, name="po2",
                                   tag="po")[0:65, :]
                    for kb in range(NKB):
                        psc = psm.tile([128, 1024], F32, name="psc", tag="pm")
                        nc.tensor.matmul(psc[:, 0:512],
                                         kp[0:64, 128 * kb:128 * kb + 128],
                                         qT_sb[p][0:64, :], start=True,
                                         stop=True)
                        nc.tensor.matmul(psc[:, 512:1024],
                                         kp[64:128, 128 * kb:128 * kb + 128],
                                         qT_sb[p][64:128, :], start=True,
                                         stop=True)
                        et = strm.tile([128, 1024], F16, name="et", tag="et",
                                       bufs=3)
                        nc.scalar.activation(et[:, :], psc[:, :], AF.Exp,
                                             scale=0.125)
                        wt = strm.tile([128, 1024], F16, name="wt", tag="wt",
                                       bufs=3)
                        delta = 128 * kb
                        ebv = eb[:, :].rearrange("p (i x) -> p i x", i=2)[
                            :, :, delta:delta + 512]
                        nc.vector.tensor_tensor(
                            wt[:, :].rearrange("p (i q) -> p i q", i=2),
                            et[:, :].rearrange("p (i q) -> p i q", i=2),
                            ebv, MULT)
                        nc.tensor.matmul(
                            po[:, :], v_sb[g][kb][:, 65 * j0:65 * j0 + 65],
                            wt[:, 0:512], start=(kb == 0),
                            stop=(kb == NKB - 1), skip_group_check=True)
                        nc.tensor.matmul(
                            po2[:, :],
                            v_sb[g][kb][:, 65 * (j0 + 1):65 * (j0 + 1) + 65],
                            wt[:, 512:1024], start=(kb == 0),
                            stop=(kb == NKB - 1), skip_group_check=True)
                        for op in slots[kb]:
                            op()
                    for hi, pot in enumerate((po, po2)):
                        # stage the denominator row into its den_sb partition
                        # (DVE copy + GpSimd SWDGE row move: the Sync queue
                        # is busy with weight streams and DMA placement is
                        # the only way to change base partition).
                        dstg = strm.tile([1, 512], F32, name="dstg",
                                         tag="dstg")
                        nc.vector.tensor_copy(dstg[:, :], pot[64:65, :])
                        if p < 6:
                            drow = 2 * p + hi
                        elif p == 6:
                            drow = 32 + hi
                        else:
                            drow = 64 + hi
                        nc.gpsimd.dma_start(
                            den_sb[drow:drow + 1, :], dstg[:, :])
                        nc.vector.tensor_copy(
                            outT_sb[p][64 * hi:64 * hi + 64, :], pot[0:64, :])
          # psm and pso closed; gpool tiles stay live.
          # pair 7's reciprocal: the only one not overlapped by attention --
          # it runs while the gate's first d=0..6 matmuls stream.
        with nc.allow_low_precision(reason="f32r reciprocal"):
            nc.vector.reciprocal(rden7[:, :], den_sb[64:66, :])

        # ============ gate + GLU ============
        # four 4-target sub-phases (4 psum banks each; 3 tags double-
        # buffered so the next sub-phase's matmuls start while the previous
        # one drains). The first sub-phase's d=0..6 matmuls overlap pair 7's
        # reciprocal; its normalization lands before the d=7 contribution.
        with tc.tile_pool(name="gate", bufs=2) as gp, \
             tc.tile_pool(name="psum_g", bufs=1, space="PSUM") as psg:
            bg_sb = gp.tile([1, 2 * D], F32R, name="bg_sb", bufs=1)
            nc.sync.dma_start(bg_sb[:, :], b_gate_d[:, :])
            first = True
            oq = 0
            for ph in range(2):
                for qh in range(2):
                    qbs = (2 * qh, 2 * qh + 1)
                    pgt = {(qb, ci): psg.tile(
                               [128, 512], F32, name=f"pg{ph}{qb}{ci}",
                               tag=f"pg{qb % 2}{ci}",
                               bufs=(1 if (qb % 2, ci) == (1, 1) else 2))
                           for qb in qbs for ci in range(2)}
                    for d in range(7):
                        for qb in qbs:
                            for ci in range(2):
                                nc.tensor.matmul(
                                    pgt[(qb, ci)],
                                    outT_sb[d][:, 128 * qb:128 * qb + 128],
                                    wg_sb[ph][:, 4096 * ci + 512 * d:
                                              4096 * ci + 512 * d + 512],
                                    start=(d == 0), stop=False,
                                    skip_group_check=True)
                    if first:
                        first = False
                        pb7 = psg.tile([128, 512], F32, name="pb7", tag="pb7")
                        nc.tensor.matmul(pb7[:, :], sel2_sb[:, :],
                                         rden7[:, :], start=True, stop=True)
                        nc.vector.tensor_tensor(
                            outT_sb[7][:, :], outT_sb[7][:, :], pb7[:, :],
                            MULT)
                    for qb in qbs:
                        for ci in range(2):
                            nc.tensor.matmul(
                                pgt[(qb, ci)],
                                outT_sb[7][:, 128 * qb:128 * qb + 128],
                                wg_sb[ph][:, 4096 * ci + 512 * 7:
                                          4096 * ci + 512 * 7 + 512],
                                start=False, stop=False,
                                skip_group_check=True)
                    for qb in qbs:
                        for ci in range(2):
                            nc.tensor.matmul(
                                pgt[(qb, ci)], ones_sb[:, :],
                                bg_sb[:, 1024 * ci + 512 * ph:
                                      1024 * ci + 512 * ph + 512],
                                start=False, stop=True, skip_group_check=True)
                    for qb in qbs:
                        sg = gp.tile([128, 512], F32, name="sg", tag="sg")
                        nc.scalar.activation(sg[:, :], pgt[(qb, 1)],
                                             AF.Sigmoid)
                        res = gp.tile([128, 512], F16, name="res", tag="res")
                        nc.vector.tensor_tensor(res[:, :], pgt[(qb, 0)],
                                                sg[:, :], MULT)
                        eng = nc.sync if oq % 2 == 0 else nc.scalar
                        oq += 1
                        eng.dma_start(
                            out_d[128 * qb:128 * qb + 128,
                                  512 * ph:512 * ph + 512],
                            res[:, :])

    nc.finalize()
    return nc


_NC_CACHE = None
_LAST_IN_MAPS = None


def _get_nc():
    global _NC_CACHE
    if _NC_CACHE is None:
        _NC_CACHE = build()
    return _NC_CACHE


def _pack_weights(w_in16, w_gate16):
    """Prepack w_in/w_gate into the exact SBUF tile layouts so every
    weight load is one full-speed contiguous-row 2D DMA."""
    wp = np.empty((128, WPACK_W), np.float16)
    # wq chunk (quad, dq): [p, 512*d2 + c] = w_in[256*dq*?? rows]
    for quad in range(2):
        for dq in range(4):
            blk = w_in16[256 * dq:256 * dq + 256,
                         2 * D + 512 * quad:2 * D + 512 * quad + 512]
            # rows (2, 128) x cols 512 -> [128, 2, 512]
            wp[:, WQ0 + 1024 * (4 * quad + dq):
               WQ0 + 1024 * (4 * quad + dq) + 1024] = \
                blk.reshape(2, 128, 512).transpose(1, 0, 2).reshape(128, 1024)
    for g in range(4):
        blk = w_in16[:, D + 256 * g:D + 256 * g + 256]
        wp[:, WV0 + 2048 * g:WV0 + 2048 * g + 2048] = \
            blk.reshape(8, 128, 256).transpose(1, 0, 2).reshape(128, 2048)
    for pp in range(8):
        blk = w_in16[:, 128 * pp:128 * pp + 128]
        wp[:, WK0 + 1024 * pp:WK0 + 1024 * pp + 1024] = \
            blk.reshape(8, 128, 128).transpose(1, 0, 2).reshape(128, 1024)
    for ph in range(2):
        for ci in range(2):
            blk = w_gate16[:, 1024 * ci + 512 * ph:1024 * ci + 512 * ph + 512]
            wp[:, WG0 + 4096 * (2 * ph + ci):
               WG0 + 4096 * (2 * ph + ci) + 4096] = \
                blk.reshape(8, 128, 512).transpose(1, 0, 2).reshape(128, 4096)
    return wp


def kernel(x, w_in, w_gate, b_gate, amplitudes, sharpness, offsets):
    x = np.ascontiguousarray(x, dtype=np.float32)
    w_in16 = np.ascontiguousarray(w_in, dtype=np.float16)
    w_gate16 = np.ascontiguousarray(w_gate, dtype=np.float16)
    b_gate = np.ascontiguousarray(b_gate, dtype=np.float32).reshape(1, 2 * D)
    amplitudes = np.asarray(amplitudes, dtype=np.float32)
    sharpness = np.asarray(sharpness, dtype=np.float32)
    offsets = np.asarray(offsets, dtype=np.float32)

    wpack = _pack_weights(w_in16, w_gate16)
    # TISA eu table computed on host (tiny O(H*K*EUW) work): eu[h, y] =
    # exp(sum_k amp * exp(-sh * (y - (off + 511 + 512r))^2)), one table per
    # query-slice index r.
    y = np.arange(EUW, dtype=np.float32)
    eus = []
    for r in range(4):
        ctr = offsets[:, :, None] + (511.0 + 512.0 * r)
        dd = y[None, None, :] - ctr
        u = (amplitudes[:, :, None] *
             np.exp(-np.abs(sharpness)[:, :, None] * dd * dd)).sum(axis=1)
        eus.append(np.exp(u).astype(np.float16))
    ones = np.ones((1, 128), np.float32)
    # sel[r, 128p + c] = 1 iff r == 2p + c//64 (head selector used to
    # broadcast the batched softmax reciprocals to 128 output rows per pair)
    sel = np.zeros((16, 1024), np.float32)
    for p_ in range(8):
        sel[2 * p_, 128 * p_:128 * p_ + 64] = 1.0
        sel[2 * p_ + 1, 128 * p_ + 64:128 * p_ + 128] = 1.0
    # sel2[r, c] = 1 iff r == c//64 (2-row selector for pairs 6/7)
    sel2 = np.zeros((2, 128), np.float32)
    sel2[0, 0:64] = 1.0
    sel2[1, 64:128] = 1.0

    in_maps = []
    for c in range(NCORES):
        b, r = c // 4, c % 4
        # kb-major pack: xT[p, 1024*kb + 128*d + c] = x[b, 128*kb+c, 128*d+p]
        xT = np.ascontiguousarray(
            x[b].reshape(16, 128, 8, 128).transpose(3, 0, 2, 1)
            .reshape(128, 16384), dtype=np.float16)
        # query columns fed in REVERSED order so the TISA bias slice per
        # k-block is an ascending (contiguous-DMA) slice of the eu table;
        # the output rows are un-reversed after the run.
        xq = np.ascontiguousarray(x[b, QS * r:QS * r + QS, :].T[:, ::-1],
                                  dtype=np.float16)
        hd = np.empty((128, 2048), np.float16)
        hd[:, 0:1024] = (xq[0:256, :].reshape(2, 128, QS)
                         .transpose(1, 0, 2).reshape(128, 1024))
        hd[:, 1024:2048] = wpack[:, WQ0:WQ0 + 1024]
        in_maps.append({
            "xT": xT, "xq": xq, "hd": hd, "wpack": wpack, "b_gate": b_gate,
            "eu": eus[r], "ones": ones, "sel": sel, "sel2": sel2,
        })

    global _LAST_IN_MAPS
    _LAST_IN_MAPS = in_maps
    nc = _get_nc()
    r_ = run_bass_kernel_spmd(nc, in_maps, core_ids=list(range(NCORES)))
    out = np.empty((B, S, D), np.float32)
    for c in range(NCORES):
        b, r = c // 4, c % 4
        out[b, QS * r:QS * r + QS, :] = \
            r_.results[c]["out"][::-1, :].astype(np.float32)
    return out
